# revision 18
# baseline (speedup 1.0000x reference)
"""Trainium2 Bass kernel for nn_DifferentiableStarPlanner.

Algorithm notes (validated bitwise vs the reference in numpy):

  * The output is exactly NUM_SWEEPS Jacobi sweeps of a 9-channel min-plus
    stencil  g <- min(g, min_c(shift_c(g) + cmap_c))  with g0 = 1e7
    everywhere except the start cell; open/close/pool never feed it.
  * Only the bounding box of the start support inflated by NUM_SWEEPS can
    change (113x113 here); everything else stays 1e7.
  * Edge-replicate padding is replaced by 1e7 guard lanes (monotone ops).
  * Per sweep only cells within t steps of the start can change, so every
    matmul and the reduce are windowed to the growing active band.

Device mapping (one NeuronCore; all 8 cores run identical replicas).
The state alternates orientation every sweep, making every neighbor shift
ONE TensorEngine transpose-mode matmul from the state. v3 batching:

  * Shifts: 3 matmul pairs per sweep (one per dx / per dy), each with a
    permutation-triplet moving operand writing 3 psum regions at once.
  * cmap preload: ONE regular fp32r matmul per sweep (identity stationary,
    moving = an SBUF tile mirroring the 9-region psum layout, center
    region zeros). fp32r halves/quarters the row cost; identity routing
    keeps cmap to ~1e-5 relative, far under the 2e-2 gate.
  * The mirror tiles are built once in setup with back-to-back batched
    transposes + two strided DVE copies (instead of 16 serialized
    matmul->copy->memset round-trips).
  * DVE runs exactly one windowed 9-region min-reduce per sweep.
"""
import sys
import os
import numpy as np

for _p in ("/opt/trn_rl_repo", "/root/.axon_site/_ro/trn_rl_repo"):
    if os.path.isdir(_p) and _p not in sys.path:
        sys.path.insert(0, _p)

import concourse.bass as bass
import concourse.bacc as bacc
import concourse.mybir as mybir
from concourse import tile
from concourse.bass_utils import run_bass_kernel_spmd

F32 = mybir.dt.float32
FR = mybir.dt.float32r
ALU = mybir.AluOpType
AXL = mybir.AxisListType
ACTF = mybir.ActivationFunctionType

INF = np.float32(1.0e7)
OC = float(np.float32(10000.0))
EPS_F = np.float32(1e-12)
NUM_SWEEPS = 80
N_CORES = 8
PRELOAD_FP32R = True

# channels: (dy, dx), center excluded
CHANNELS = [(dy, dx) for dy in (-1, 0, 1) for dx in (-1, 0, 1) if not (dy == 0 and dx == 0)]
SS = 116  # psum region stride within a bank
NUM_FILLERS = 3
FILLER_N = 32


def build_program(Dr, Dc, seed_rlo, seed_rhi, seed_clo, seed_chi, r0, c0,
                  H, W, num_sweeps):
    """Domain = grid rows r0..r0+Dr-1, cols c0..c0+Dc-1; seed_* in domain coords."""
    Sr, Sc = Dr + 2, Dc + 2
    KR, KC = Dr + 1, Dc + 1      # state partition counts incl junk/guard lane
    assert KR <= 115 and KC <= 115 and Sc <= 128 and 3 * SS <= 1536

    nc = bacc.Bacc("TRN2", target_bir_lowering=False, debug=False)

    # ---- DRAM I/O (inputs packed: single DMA) ----
    seg = [("obsT", Sr), ("obsTm", Sr), ("obsTp", Sr), ("xcT", Sr), ("xcTm", Sr),
           ("xcTp", Sr), ("ycT", Sr), ("startm", Dc), ("ident", Sc), ("sig", Sc),
           ("cr3", 3 * KR), ("cc3", 3 * KC)]
    offs, TOT = {}, 0
    for nm, wd in seg:
        offs[nm] = TOT
        TOT += wd
    d_pack = nc.dram_tensor("packed", [Sc, TOT], F32, kind="ExternalInput")
    d_out = nc.dram_tensor("out", [H, W], F32, kind="ExternalOutput")

    with tile.TileContext(nc) as tc:
        from contextlib import ExitStack
        with ExitStack() as ctx:
            sb = ctx.enter_context(tc.tile_pool(name="sb", bufs=1))
            ps = ctx.enter_context(tc.tile_pool(name="ps", bufs=1, space="PSUM"))

            # ---- SBUF tiles ----
            t_all = sb.tile([Sc, TOT], F32)
            t_in = {nm: t_all[:, offs[nm]:offs[nm] + Sr] for nm in
                    ("obsT", "obsTm", "obsTp", "xcT", "xcTm", "xcTp", "ycT")}
            t_start = t_all[0:Dr, offs["startm"]:offs["startm"] + Dc]
            # DVE-owned copies of the constant matrices
            identC = sb.tile([Sc, Sc], F32)
            sigC = sb.tile([Sc, Sc], F32)
            cr3C = sb.tile([KR, 3 * KR], F32)
            cc3C = sb.tile([KC, 3 * KC], F32)
            g_rm = sb.tile([KR, Dc + 3], F32)   # rows+junk | colguard,cols,2 guards
            s_T = sb.tile([KC, Dr + 3], F32)    # cols+junk | rowguard,rows,2 guards
            bg = sb.tile([128, W], F32)
            bias_eps = sb.tile([Sc, 1], F32)
            sq = {nm: sb.tile([Sc, Dr], F32, name=f"sq_{nm}") for nm in ("L", "R", "U", "D")}
            t_tmp = sb.tile([Sc, Dr], F32)
            t_A = {ch: sb.tile([Sc, Dr], F32, name=f"A_{ch[0]+1}{ch[1]+1}") for ch in CHANNELS}
            t_mx = {ch: sb.tile([Sc, Dr], F32, name=f"mx_{ch[0]+1}{ch[1]+1}") for ch in CHANNELS}
            # transposed cmap (+1e7 junk-row slot); partitions = cols -1..Dc
            t_cmapT = {ch: sb.tile([Sc, KR], F32, name=f"cmapT_{ch[0]+1}{ch[1]+1}")
                       for ch in CHANNELS}
            # psum-layout mirrors of the per-phase cmap preloads
            cmapA_all = sb.tile([128, 1536], F32)
            cmapB_all = sb.tile([128, 1536], F32)

            # ---- PSUM: two bank sets of 3 banks (3 regions each) ----
            psum_sets = [ps.tile([128, 1536], F32, name="psumA"),
                         ps.tile([128, 1536], F32, name="psumB")]
            psD = ps.tile([128, 512], F32, name="psD")
            t_warm = sb.tile([128, 512], mybir.dt.bfloat16)

            # ---- load inputs (single DMA) + const copies ----
            nc.sync.dma_start(t_all[:], d_pack.ap())
            v = nc.vector
            v.tensor_copy(identC[:], t_all[:, offs["ident"]:offs["ident"] + Sc])
            v.tensor_copy(sigC[:], t_all[:, offs["sig"]:offs["sig"] + Sc])
            v.tensor_copy(cr3C[:], t_all[0:KR, offs["cr3"]:offs["cr3"] + 3 * KR])
            v.tensor_copy(cc3C[:], t_all[0:KC, offs["cc3"]:offs["cc3"] + 3 * KC])

            # ---- init ----
            v.memset(t_warm[:], 1.0)
            v.memset(bg[:], INF)
            v.memset(g_rm[:], INF)
            v.memset(s_T[:], INF)
            v.memset(bias_eps[:], EPS_F)

            # ---- background writes (1e7 outside the domain) ----
            out_ap = d_out.ap()
            bg_rows = []
            if r0 > 0:
                bg_rows.append((0, r0))
            if r0 + Dr < H:
                bg_rows.append((r0 + Dr, H))
            for lo_, hi_ in bg_rows:
                r = lo_
                while r < hi_:
                    n = min(128, hi_ - r)
                    nc.sync.dma_start(out_ap[r:r + n, :], bg[0:n, :])
                    r += n
            if c0 > 0:
                nc.sync.dma_start(out_ap[r0:r0 + Dr, 0:c0], bg[0:Dr, 0:c0])
            if c0 + Dc < W:
                nc.sync.dma_start(out_ap[r0:r0 + Dr, c0 + Dc:W],
                                  bg[0:Dr, 0:W - c0 - Dc])

            # ---- cmap channels, computed in transposed orientation ----
            rows = slice(1, 1 + Dr)
            v.tensor_sub(t_tmp[:], t_in["xcT"][:, rows], t_in["xcTm"][:, rows])
            v.tensor_mul(sq["L"][:], t_tmp[:], t_tmp[:])
            v.tensor_sub(t_tmp[:], t_in["xcT"][:, rows], t_in["xcTp"][:, rows])
            v.tensor_mul(sq["R"][:], t_tmp[:], t_tmp[:])
            v.tensor_sub(t_tmp[:], t_in["ycT"][:, rows], t_in["ycT"][:, 2:2 + Dr])
            v.tensor_mul(sq["U"][:], t_tmp[:], t_tmp[:])
            v.tensor_sub(t_tmp[:], t_in["ycT"][:, rows], t_in["ycT"][:, 0:Dr])
            v.tensor_mul(sq["D"][:], t_tmp[:], t_tmp[:])

            geo = {(-1, -1): ("L", "U"), (0, -1): ("L",), (1, -1): ("L", "D"),
                   (-1, 0): ("U",), (1, 0): ("D",),
                   (-1, 1): ("R", "U"), (0, 1): ("R",), (1, 1): ("R", "D")}
            obsnb = {(-1, -1): (-1, -1), (0, -1): (-1, 0), (1, -1): (1, -1),
                     (-1, 0): (-1, 0), (1, 0): (1, 0),
                     (-1, 1): (-1, 1), (0, 1): (0, 1), (1, 1): (1, 1)}
            obs_by_dx = {-1: "obsTm", 0: "obsT", 1: "obsTp"}
            for ch in CHANNELS:
                terms = geo[ch]
                if len(terms) == 2:
                    v.tensor_add(t_A[ch][:], sq[terms[0]][:], sq[terms[1]][:])
                    nc.scalar.activation(t_A[ch][:], t_A[ch][:], ACTF.Sqrt,
                                         bias=bias_eps[:], scale=1.0)
                else:
                    nc.scalar.activation(t_A[ch][:], sq[terms[0]][:], ACTF.Sqrt,
                                         bias=bias_eps[:], scale=1.0)
                ody, odx = obsnb[ch]
                nbt = t_in[obs_by_dx[odx]]
                v.tensor_max(t_mx[ch][:], nbt[:, 1 + ody:1 + ody + Dr],
                             t_in["obsT"][:, rows])
                # junk-row slot (free index Dr) to 1e7 first, channels to 0..Dr-1
                v.memset(t_cmapT[ch][:, Dr:KR], INF)
                v.scalar_tensor_tensor(t_cmapT[ch][:, 0:Dr], t_mx[ch][:], OC,
                                       t_A[ch][:], op0=ALU.mult, op1=ALU.add)
                # col -1 lane to 1e7 (used as the junk-col source via sig)
                v.memset(t_cmapT[ch][0:1, :], INF)

            # ---- helpers ----
            def apx(base, elem_off, dims):
                pap = list(base.ap)
                return bass.AP(base.tensor, base.offset + elem_off,
                               [list(pap[0])] + [list(d) for d in dims])

            # ---- row-major cmap: 8 batched sig-transposes -> psum -> 1 copy
            # (cmapB_all holds the 8 channels at stride SS; junk lanes come
            # straight from the sig transpose's 1e7 source lane)
            for i, ch in enumerate(CHANNELS):
                nc.tensor.matmul(psum_sets[1][0:KR, i * SS:i * SS + Sc],
                                 lhsT=t_cmapT[ch][:], rhs=sigC[:],
                                 is_transpose=True, start=True, stop=True,
                                 skip_group_check=True)
            v.tensor_copy(apx(cmapB_all[0:KR, 0:1536], 0, [[SS, 8], [1, KC]]),
                          apx(psum_sets[1][0:KR, 0:1536], 0, [[SS, 8], [1, KC]]))

            # ---- g0 = clip(INF*(1-start), 0, INF) ----
            v.tensor_scalar(g_rm[0:Dr, 1:1 + Dc], t_start[:], -float(INF), float(INF),
                            op0=ALU.mult, op1=ALU.add)
            v.tensor_scalar_max(g_rm[0:Dr, 1:1 + Dc], g_rm[0:Dr, 1:1 + Dc], 0.0)

            def win_rows(t):
                return max(0, seed_rlo - t), min(Dr - 1, seed_rhi + t)

            def win_cols(t):
                return max(0, seed_clo - t), min(Dc - 1, seed_chi + t)

            def preload_A(set_idx, lo, hi):
                # cmap for an odd (g_rm -> s_T) sweep: column-major psum layout
                for dy in (-1, 0, 1):
                    first = True
                    for dx in (-1, 0, 1):
                        if dy == 0 and dx == 0:
                            continue
                        i = CHANNELS.index((dy, dx))
                        off = (dy + 1) * 512 + (dx + 1) * SS
                        nc.tensor.matmul(
                            psum_sets[set_idx][0:KC, off + lo:off + hi + 1],
                            lhsT=cmapB_all[0:KR, i * SS:i * SS + KC],
                            rhs=identC[0:KR, lo:hi + 1],
                            is_transpose=True, start=first, stop=False)
                        first = False

            def preload_B(set_idx, lo, hi):
                # cmap for an even (s_T -> g_rm) sweep: row-major psum layout
                for dy in (-1, 0, 1):
                    first = True
                    for dx in (-1, 0, 1):
                        if dy == 0 and dx == 0:
                            continue
                        off = (dy + 1) * 512 + (dx + 1) * SS
                        nc.tensor.matmul(
                            psum_sets[set_idx][0:KR, off + lo:off + hi + 1],
                            lhsT=t_cmapT[(dy, dx)][:, 0:KR],
                            rhs=sigC[:, lo:hi + 1],
                            is_transpose=True, start=first, stop=False)
                        first = False

            preload_A(0, *win_rows(1))

            # ---- sweeps ----
            for t in range(1, num_sweeps + 1):
                cur = psum_sets[(t - 1) % 2]
                if t % 2 == 1:
                    # phase A: g_rm -> s_T; windowed over rows
                    lo, hi = win_rows(t)
                    for dx in (-1, 0, 1):
                        for dy in (-1, 0, 1):
                            off = (dy + 1) * 512 + (dx + 1) * SS
                            rhs = cr3C[0:KR, (dy + 1) * KR + lo:
                                       (dy + 1) * KR + hi + 1]
                            nc.tensor.matmul(
                                cur[0:KC, off + lo:off + hi + 1],
                                lhsT=g_rm[:, (1 + dx):(1 + dx) + KC],
                                rhs=rhs,
                                is_transpose=True, start=False, stop=(dx == 1))
                    in_ap = apx(cur[0:KC, 0:1536], lo, [[1, hi - lo + 1], [512, 3], [SS, 3]])
                    v.tensor_reduce(s_T[:, 1 + lo:1 + hi + 1], in_ap,
                                    axis=AXL.XY, op=ALU.min)
                else:
                    # phase B: s_T -> g_rm; windowed over cols
                    lo, hi = win_cols(t)
                    for dy in (-1, 0, 1):
                        for dx in (-1, 0, 1):
                            off = (dy + 1) * 512 + (dx + 1) * SS
                            rhs = cc3C[0:KC, (dx + 1) * KC + lo:
                                       (dx + 1) * KC + hi + 1]
                            nc.tensor.matmul(
                                cur[0:KR, off + lo:off + hi + 1],
                                lhsT=s_T[:, (1 + dy):(1 + dy) + KR],
                                rhs=rhs,
                                is_transpose=True, start=False, stop=(dx == 1))
                    in_ap = apx(cur[0:KR, 0:1536], lo, [[1, hi - lo + 1], [512, 3], [SS, 3]])
                    v.tensor_reduce(g_rm[:, 1 + lo:1 + hi + 1], in_ap,
                                    axis=AXL.XY, op=ALU.min)

                if t < num_sweeps:
                    if t % 2 == 1:
                        preload_B(t % 2, *win_cols(t + 1))
                    else:
                        preload_A(t % 2, *win_rows(t + 1))
                    # PE fillers: keep the pipeline from draining in the idle
                    # gap between the preload and the next sweep's shifts.
                    for _ in range(NUM_FILLERS):
                        nc.tensor.matmul(psD[0:FILLER_N, 0:FILLER_N],
                                         lhsT=identC[0:FILLER_N, 0:FILLER_N],
                                         rhs=identC[0:FILLER_N, 0:FILLER_N],
                                         is_transpose=True,
                                         start=True, stop=True,
                                         skip_group_check=True)

            # ---- final state to row-major if needed, then write out ----
            if num_sweeps % 2 == 1:
                fin = psum_sets[num_sweeps % 2][0:KR, 0:KC]
                nc.tensor.matmul(fin, lhsT=s_T[:, 1:1 + KR],
                                 rhs=identC[0:KC, 0:KC],
                                 is_transpose=True, start=True, stop=True)
                v.tensor_copy(g_rm[0:Dr, 1:1 + Dc], fin[0:Dr, 0:Dc])
            nc.sync.dma_start(out_ap[r0:r0 + Dr, c0:c0 + Dc], g_rm[0:Dr, 1:1 + Dc])

    nc.compile()
    return nc, ["packed"]


def prep_inputs(obstacles, coords, start_map, num_sweeps=NUM_SWEEPS):
    """Host-side slicing/layout prep. Returns (in_map, geometry)."""
    obs = np.asarray(obstacles, np.float32)[0, 0]
    yc = np.asarray(coords, np.float32)[0, 0]
    xc = np.asarray(coords, np.float32)[0, 1]
    s = np.asarray(start_map, np.float32)[0, 0]
    H, W = obs.shape

    ys, xs = np.nonzero(s > 0)
    assert len(ys) >= 1, "empty start_map"
    r0 = max(0, int(ys.min()) - num_sweeps)
    r1 = min(H - 1, int(ys.max()) + num_sweeps)
    c0 = max(0, int(xs.min()) - num_sweeps)
    c1 = min(W - 1, int(xs.max()) + num_sweeps)
    Dr, Dc = r1 - r0 + 1, c1 - c0 + 1
    Sr, Sc = Dr + 2, Dc + 2
    KR, KC = Dr + 1, Dc + 1

    def pad_slice(a):
        ap = np.pad(a, 1, mode='edge')
        return np.ascontiguousarray(ap[r0:r0 + Sr, c0:c0 + Sc], dtype=np.float32)

    obs_p, yc_p, xc_p = pad_slice(obs), pad_slice(yc), pad_slice(xc)

    def tsh(a, dx):
        at = np.ascontiguousarray(a.T)
        if dx == 0:
            return at
        out = np.empty_like(at)
        if dx == -1:
            out[1:] = at[:-1]
            out[0] = at[0]
        else:
            out[:-1] = at[1:]
            out[-1] = at[-1]
        return out

    def cyc(n, d):
        # P[k, j] = 1 iff k == (j + d) mod n
        P = np.zeros((n, n), np.float32)
        P[(np.arange(n) + d) % n, np.arange(n)] = 1.0
        return P

    # sig: out free slot j <- cmapT partition sigma(j);
    # sigma(j) = j+1 for real cols, junk-col slot Dc -> partition 0 (1e7 lane)
    sigma = np.concatenate([np.arange(1, Sc), [0]])
    sigma[Dc] = 0
    sigma[Sc - 1] = Dc + 1
    assert sorted(sigma.tolist()) == list(range(Sc))
    sig = np.zeros((Sc, Sc), np.float32)
    sig[sigma, np.arange(Sc)] = 1.0

    def frame(a, pw):
        out = np.zeros((Sc, pw), np.float32)
        out[0:a.shape[0], 0:a.shape[1]] = a
        return out

    startm = np.zeros((Sc, Dc), np.float32)
    startm[0:Dr, :] = s[r0:r1 + 1, c0:c1 + 1]
    cr3 = np.concatenate([cyc(KR, -1), np.eye(KR, dtype=np.float32),
                          cyc(KR, 1)], axis=1)
    cc3 = np.concatenate([cyc(KC, -1), np.eye(KC, dtype=np.float32),
                          cyc(KC, 1)], axis=1)
    packed = np.concatenate([
        tsh(obs_p, 0), tsh(obs_p, -1), tsh(obs_p, 1),
        tsh(xc_p, 0), tsh(xc_p, -1), tsh(xc_p, 1), tsh(yc_p, 0),
        startm, np.eye(Sc, dtype=np.float32), sig,
        frame(cr3, 3 * KR), frame(cc3, 3 * KC),
    ], axis=1)
    in_map = {"packed": np.ascontiguousarray(packed, dtype=np.float32)}

    geom = dict(Dr=Dr, Dc=Dc, r0=r0, c0=c0, H=H, W=W,
                seed_rlo=int(ys.min()) - r0, seed_rhi=int(ys.max()) - r0,
                seed_clo=int(xs.min()) - c0, seed_chi=int(xs.max()) - c0)
    return in_map, geom


def kernel(obstacles, coords, start_map, goal_map):
    in_map, gm = prep_inputs(obstacles, coords, start_map)
    nc, _ = build_program(gm["Dr"], gm["Dc"], gm["seed_rlo"], gm["seed_rhi"],
                          gm["seed_clo"], gm["seed_chi"], gm["r0"], gm["c0"],
                          gm["H"], gm["W"], NUM_SWEEPS)
    in_maps = [in_map for _ in range(N_CORES)]
    res = run_bass_kernel_spmd(nc, in_maps, core_ids=list(range(N_CORES)))
    out = res.results[0]["out"]
    return np.ascontiguousarray(out[None, None]).astype(np.float32)


# revision 19
# speedup vs baseline: 1.1603x; 1.1603x over previous
"""Trainium2 Bass kernel for nn_DifferentiableStarPlanner.

Algorithm notes (validated bitwise vs the reference in numpy):

  * The output is exactly NUM_SWEEPS Jacobi sweeps of a 9-channel min-plus
    stencil  g <- min(g, min_c(shift_c(g) + cmap_c))  with g0 = 1e7
    everywhere except the start cell; open/close/pool never feed it.
  * Only the bounding box of the start support inflated by NUM_SWEEPS can
    change (113x113 here); everything else stays 1e7.
  * Edge-replicate padding is replaced by 1e7 guard lanes (monotone ops).
  * Per sweep only cells within t steps of the start can change, so every
    matmul and the reduce are windowed to the growing active band.

Device mapping (one NeuronCore; all 8 cores run identical replicas).
The state alternates orientation every sweep, making every neighbor shift
ONE TensorEngine transpose-mode matmul from the state. v3 batching:

  * Shifts: 3 matmul pairs per sweep (one per dx / per dy), each with a
    permutation-triplet moving operand writing 3 psum regions at once.
  * cmap preload: ONE regular fp32r matmul per sweep (identity stationary,
    moving = an SBUF tile mirroring the 9-region psum layout, center
    region zeros). fp32r halves/quarters the row cost; identity routing
    keeps cmap to ~1e-5 relative, far under the 2e-2 gate.
  * The mirror tiles are built once in setup with back-to-back batched
    transposes + two strided DVE copies (instead of 16 serialized
    matmul->copy->memset round-trips).
  * DVE runs exactly one windowed 9-region min-reduce per sweep.
"""
import sys
import os
import numpy as np

for _p in ("/opt/trn_rl_repo", "/root/.axon_site/_ro/trn_rl_repo"):
    if os.path.isdir(_p) and _p not in sys.path:
        sys.path.insert(0, _p)

import concourse.bass as bass
import concourse.bacc as bacc
import concourse.mybir as mybir
from concourse import tile
from concourse.bass_utils import run_bass_kernel_spmd

F32 = mybir.dt.float32
FR = mybir.dt.float32r
ALU = mybir.AluOpType
AXL = mybir.AxisListType
ACTF = mybir.ActivationFunctionType

INF = np.float32(1.0e7)
OC = float(np.float32(10000.0))
EPS_F = np.float32(1e-12)
NUM_SWEEPS = 80
N_CORES = 8
PRELOAD_FP32R = True

# channels: (dy, dx), center excluded
CHANNELS = [(dy, dx) for dy in (-1, 0, 1) for dx in (-1, 0, 1) if not (dy == 0 and dx == 0)]
SS = 116  # psum region stride within a bank
NUM_FILLERS = 3
FILLER_N = 32


def build_program(Dr, Dc, seed_rlo, seed_rhi, seed_clo, seed_chi, r0, c0,
                  H, W, num_sweeps):
    """Domain = grid rows r0..r0+Dr-1, cols c0..c0+Dc-1; seed_* in domain coords."""
    Sr, Sc = Dr + 2, Dc + 2
    KR, KC = Dr + 1, Dc + 1      # state partition counts incl junk/guard lane
    assert KR <= 115 and KC <= 115 and Sc <= 128 and 3 * SS <= 1536

    nc = bacc.Bacc("TRN2", target_bir_lowering=False, debug=False)

    # ---- DRAM I/O (inputs packed: single DMA) ----
    seg = [("obsT", Sr), ("obsTm", Sr), ("obsTp", Sr), ("xcT", Sr), ("xcTm", Sr),
           ("xcTp", Sr), ("ycT", Sr), ("startm", Dc), ("ident", Sc), ("sig", Sc),
           ("cr3", 3 * KR), ("cc3", 3 * KC)]
    offs, TOT = {}, 0
    for nm, wd in seg:
        offs[nm] = TOT
        TOT += wd
    d_pack = nc.dram_tensor("packed", [Sc, TOT], F32, kind="ExternalInput")
    d_out = nc.dram_tensor("out", [H, W], F32, kind="ExternalOutput")

    with tile.TileContext(nc) as tc:
        from contextlib import ExitStack
        with ExitStack() as ctx:
            sb = ctx.enter_context(tc.tile_pool(name="sb", bufs=1))
            ps = ctx.enter_context(tc.tile_pool(name="ps", bufs=1, space="PSUM"))

            # ---- SBUF tiles ----
            t_all = sb.tile([Sc, TOT], F32)
            t_in = {nm: t_all[:, offs[nm]:offs[nm] + Sr] for nm in
                    ("obsT", "obsTm", "obsTp", "xcT", "xcTm", "xcTp", "ycT")}
            t_start = t_all[0:Dr, offs["startm"]:offs["startm"] + Dc]
            # DVE-owned copies of the constant matrices
            identC = sb.tile([Sc, Sc], F32)
            sigC = sb.tile([Sc, Sc], F32)
            cr3C = sb.tile([KR, 3 * KR], F32)
            cc3C = sb.tile([KC, 3 * KC], F32)
            g_rm = sb.tile([KR, Dc + 3], F32)   # rows+junk | colguard,cols,2 guards
            s_T = sb.tile([KC, Dr + 3], F32)    # cols+junk | rowguard,rows,2 guards
            bg = sb.tile([128, W], F32)
            bias_eps = sb.tile([Sc, 1], F32)
            sq = {nm: sb.tile([Sc, Dr], F32, name=f"sq_{nm}") for nm in ("L", "R", "U", "D")}
            t_tmp = sb.tile([Sc, Dr], F32)
            t_A = {ch: sb.tile([Sc, Dr], F32, name=f"A_{ch[0]+1}{ch[1]+1}") for ch in CHANNELS}
            t_mx = {ch: sb.tile([Sc, Dr], F32, name=f"mx_{ch[0]+1}{ch[1]+1}") for ch in CHANNELS}
            # transposed cmap (+1e7 junk-row slot); partitions = cols -1..Dc
            t_cmapT = {ch: sb.tile([Sc, KR], F32, name=f"cmapT_{ch[0]+1}{ch[1]+1}")
                       for ch in CHANNELS}
            # psum-layout mirrors of the per-phase cmap preloads
            cmapA_all = sb.tile([128, 1536], F32)
            cmapB_all = sb.tile([128, 1536], F32)

            # ---- PSUM: two bank sets of 3 banks (3 regions each) ----
            psum_sets = [ps.tile([128, 1536], F32, name="psumA"),
                         ps.tile([128, 1536], F32, name="psumB")]
            psD = ps.tile([128, 512], F32, name="psD")
            t_warm = sb.tile([128, 512], mybir.dt.bfloat16)

            # ---- load inputs (single DMA) + const copies ----
            nc.sync.dma_start(t_all[:], d_pack.ap())
            v = nc.vector
            v.tensor_copy(identC[:], t_all[:, offs["ident"]:offs["ident"] + Sc])
            v.tensor_copy(sigC[:], t_all[:, offs["sig"]:offs["sig"] + Sc])
            v.tensor_copy(cr3C[:], t_all[0:KR, offs["cr3"]:offs["cr3"] + 3 * KR])
            v.tensor_copy(cc3C[:], t_all[0:KC, offs["cc3"]:offs["cc3"] + 3 * KC])

            # ---- init ----
            v.memset(t_warm[:], 1.0)
            v.memset(bg[:], INF)
            v.memset(g_rm[:], INF)
            v.memset(s_T[:], INF)
            v.memset(bias_eps[:], EPS_F)

            # ---- background writes (1e7 outside the domain) ----
            out_ap = d_out.ap()
            bg_rows = []
            if r0 > 0:
                bg_rows.append((0, r0))
            if r0 + Dr < H:
                bg_rows.append((r0 + Dr, H))
            for lo_, hi_ in bg_rows:
                r = lo_
                while r < hi_:
                    n = min(128, hi_ - r)
                    nc.sync.dma_start(out_ap[r:r + n, :], bg[0:n, :])
                    r += n
            if c0 > 0:
                nc.sync.dma_start(out_ap[r0:r0 + Dr, 0:c0], bg[0:Dr, 0:c0])
            if c0 + Dc < W:
                nc.sync.dma_start(out_ap[r0:r0 + Dr, c0 + Dc:W],
                                  bg[0:Dr, 0:W - c0 - Dc])

            # ---- cmap channels, computed in transposed orientation ----
            rows = slice(1, 1 + Dr)
            v.tensor_sub(t_tmp[:], t_in["xcT"][:, rows], t_in["xcTm"][:, rows])
            v.tensor_mul(sq["L"][:], t_tmp[:], t_tmp[:])
            v.tensor_sub(t_tmp[:], t_in["xcT"][:, rows], t_in["xcTp"][:, rows])
            v.tensor_mul(sq["R"][:], t_tmp[:], t_tmp[:])
            v.tensor_sub(t_tmp[:], t_in["ycT"][:, rows], t_in["ycT"][:, 2:2 + Dr])
            v.tensor_mul(sq["U"][:], t_tmp[:], t_tmp[:])
            v.tensor_sub(t_tmp[:], t_in["ycT"][:, rows], t_in["ycT"][:, 0:Dr])
            v.tensor_mul(sq["D"][:], t_tmp[:], t_tmp[:])

            geo = {(-1, -1): ("L", "U"), (0, -1): ("L",), (1, -1): ("L", "D"),
                   (-1, 0): ("U",), (1, 0): ("D",),
                   (-1, 1): ("R", "U"), (0, 1): ("R",), (1, 1): ("R", "D")}
            obsnb = {(-1, -1): (-1, -1), (0, -1): (-1, 0), (1, -1): (1, -1),
                     (-1, 0): (-1, 0), (1, 0): (1, 0),
                     (-1, 1): (-1, 1), (0, 1): (0, 1), (1, 1): (1, 1)}
            obs_by_dx = {-1: "obsTm", 0: "obsT", 1: "obsTp"}
            for ch in CHANNELS:
                terms = geo[ch]
                if len(terms) == 2:
                    v.tensor_add(t_A[ch][:], sq[terms[0]][:], sq[terms[1]][:])
                    nc.scalar.activation(t_A[ch][:], t_A[ch][:], ACTF.Sqrt,
                                         bias=bias_eps[:], scale=1.0)
                else:
                    nc.scalar.activation(t_A[ch][:], sq[terms[0]][:], ACTF.Sqrt,
                                         bias=bias_eps[:], scale=1.0)
                ody, odx = obsnb[ch]
                nbt = t_in[obs_by_dx[odx]]
                v.tensor_max(t_mx[ch][:], nbt[:, 1 + ody:1 + ody + Dr],
                             t_in["obsT"][:, rows])
                # junk-row slot (free index Dr) to 1e7 first, channels to 0..Dr-1
                v.memset(t_cmapT[ch][:, Dr:KR], INF)
                v.scalar_tensor_tensor(t_cmapT[ch][:, 0:Dr], t_mx[ch][:], OC,
                                       t_A[ch][:], op0=ALU.mult, op1=ALU.add)
                # col -1 lane to 1e7 (used as the junk-col source via sig)
                v.memset(t_cmapT[ch][0:1, :], INF)

            # ---- helpers ----
            def apx(base, elem_off, dims):
                pap = list(base.ap)
                return bass.AP(base.tensor, base.offset + elem_off,
                               [list(pap[0])] + [list(d) for d in dims])

            # ---- PE warm-up: keep the clock controller fed while the DVE
            # builds cmap (the high-activity clock boost engages based on
            # sustained tensor-engine activity)
            for _ in range(40):
                nc.tensor.matmul(psD[0:Sc, 0:Sc], lhsT=identC[:],
                                 rhs=identC[:], is_transpose=True,
                                 start=True, stop=True, skip_group_check=True)

            # ---- row-major cmap: 8 batched sig-transposes -> psum -> 1 copy
            # (cmapB_all holds the 8 channels at stride SS; junk lanes come
            # straight from the sig transpose's 1e7 source lane)
            for i, ch in enumerate(CHANNELS):
                nc.tensor.matmul(psum_sets[1][0:KR, i * SS:i * SS + Sc],
                                 lhsT=t_cmapT[ch][:], rhs=sigC[:],
                                 is_transpose=True, start=True, stop=True,
                                 skip_group_check=True)
            v.tensor_copy(apx(cmapB_all[0:KR, 0:1536], 0, [[SS, 8], [1, KC]]),
                          apx(psum_sets[1][0:KR, 0:1536], 0, [[SS, 8], [1, KC]]))

            # ---- g0 = clip(INF*(1-start), 0, INF) ----
            v.tensor_scalar(g_rm[0:Dr, 1:1 + Dc], t_start[:], -float(INF), float(INF),
                            op0=ALU.mult, op1=ALU.add)
            v.tensor_scalar_max(g_rm[0:Dr, 1:1 + Dc], g_rm[0:Dr, 1:1 + Dc], 0.0)

            def win_rows(t):
                return max(0, seed_rlo - t), min(Dr - 1, seed_rhi + t)

            def win_cols(t):
                return max(0, seed_clo - t), min(Dc - 1, seed_chi + t)

            def preload_A(set_idx, lo, hi):
                # cmap for an odd (g_rm -> s_T) sweep: column-major psum layout
                for dy in (-1, 0, 1):
                    first = True
                    for dx in (-1, 0, 1):
                        if dy == 0 and dx == 0:
                            continue
                        i = CHANNELS.index((dy, dx))
                        off = (dy + 1) * 512 + (dx + 1) * SS
                        nc.tensor.matmul(
                            psum_sets[set_idx][0:KC, off + lo:off + hi + 1],
                            lhsT=cmapB_all[0:KR, i * SS:i * SS + KC],
                            rhs=identC[0:KR, lo:hi + 1],
                            is_transpose=True, start=first, stop=False)
                        first = False

            def preload_B(set_idx, lo, hi):
                # cmap for an even (s_T -> g_rm) sweep: row-major psum layout
                for dy in (-1, 0, 1):
                    first = True
                    for dx in (-1, 0, 1):
                        if dy == 0 and dx == 0:
                            continue
                        off = (dy + 1) * 512 + (dx + 1) * SS
                        nc.tensor.matmul(
                            psum_sets[set_idx][0:KR, off + lo:off + hi + 1],
                            lhsT=t_cmapT[(dy, dx)][:, 0:KR],
                            rhs=sigC[:, lo:hi + 1],
                            is_transpose=True, start=first, stop=False)
                        first = False

            preload_A(0, *win_rows(1))

            # ---- sweeps ----
            for t in range(1, num_sweeps + 1):
                cur = psum_sets[(t - 1) % 2]
                if t % 2 == 1:
                    # phase A: g_rm -> s_T; windowed over rows
                    lo, hi = win_rows(t)
                    for dx in (-1, 0, 1):
                        for dy in (-1, 0, 1):
                            off = (dy + 1) * 512 + (dx + 1) * SS
                            rhs = cr3C[0:KR, (dy + 1) * KR + lo:
                                       (dy + 1) * KR + hi + 1]
                            nc.tensor.matmul(
                                cur[0:KC, off + lo:off + hi + 1],
                                lhsT=g_rm[:, (1 + dx):(1 + dx) + KC],
                                rhs=rhs,
                                is_transpose=True, start=False, stop=(dx == 1))
                    in_ap = apx(cur[0:KC, 0:1536], lo, [[1, hi - lo + 1], [512, 3], [SS, 3]])
                    v.tensor_reduce(s_T[:, 1 + lo:1 + hi + 1], in_ap,
                                    axis=AXL.XY, op=ALU.min)
                else:
                    # phase B: s_T -> g_rm; windowed over cols
                    lo, hi = win_cols(t)
                    for dy in (-1, 0, 1):
                        for dx in (-1, 0, 1):
                            off = (dy + 1) * 512 + (dx + 1) * SS
                            rhs = cc3C[0:KC, (dx + 1) * KC + lo:
                                       (dx + 1) * KC + hi + 1]
                            nc.tensor.matmul(
                                cur[0:KR, off + lo:off + hi + 1],
                                lhsT=s_T[:, (1 + dy):(1 + dy) + KR],
                                rhs=rhs,
                                is_transpose=True, start=False, stop=(dx == 1))
                    in_ap = apx(cur[0:KR, 0:1536], lo, [[1, hi - lo + 1], [512, 3], [SS, 3]])
                    v.tensor_reduce(g_rm[:, 1 + lo:1 + hi + 1], in_ap,
                                    axis=AXL.XY, op=ALU.min)

                if t < num_sweeps:
                    if t % 2 == 1:
                        preload_B(t % 2, *win_cols(t + 1))
                    else:
                        preload_A(t % 2, *win_rows(t + 1))
                    # PE fillers: keep the pipeline from draining in the idle
                    # gap between the preload and the next sweep's shifts.
                    for _ in range(NUM_FILLERS):
                        nc.tensor.matmul(psD[0:FILLER_N, 0:FILLER_N],
                                         lhsT=identC[0:FILLER_N, 0:FILLER_N],
                                         rhs=identC[0:FILLER_N, 0:FILLER_N],
                                         is_transpose=True,
                                         start=True, stop=True,
                                         skip_group_check=True)

            # ---- final state to row-major if needed, then write out ----
            if num_sweeps % 2 == 1:
                fin = psum_sets[num_sweeps % 2][0:KR, 0:KC]
                nc.tensor.matmul(fin, lhsT=s_T[:, 1:1 + KR],
                                 rhs=identC[0:KC, 0:KC],
                                 is_transpose=True, start=True, stop=True)
                v.tensor_copy(g_rm[0:Dr, 1:1 + Dc], fin[0:Dr, 0:Dc])
            nc.sync.dma_start(out_ap[r0:r0 + Dr, c0:c0 + Dc], g_rm[0:Dr, 1:1 + Dc])

    nc.compile()
    return nc, ["packed"]


def prep_inputs(obstacles, coords, start_map, num_sweeps=NUM_SWEEPS):
    """Host-side slicing/layout prep. Returns (in_map, geometry)."""
    obs = np.asarray(obstacles, np.float32)[0, 0]
    yc = np.asarray(coords, np.float32)[0, 0]
    xc = np.asarray(coords, np.float32)[0, 1]
    s = np.asarray(start_map, np.float32)[0, 0]
    H, W = obs.shape

    ys, xs = np.nonzero(s > 0)
    assert len(ys) >= 1, "empty start_map"
    r0 = max(0, int(ys.min()) - num_sweeps)
    r1 = min(H - 1, int(ys.max()) + num_sweeps)
    c0 = max(0, int(xs.min()) - num_sweeps)
    c1 = min(W - 1, int(xs.max()) + num_sweeps)
    Dr, Dc = r1 - r0 + 1, c1 - c0 + 1
    Sr, Sc = Dr + 2, Dc + 2
    KR, KC = Dr + 1, Dc + 1

    def pad_slice(a):
        ap = np.pad(a, 1, mode='edge')
        return np.ascontiguousarray(ap[r0:r0 + Sr, c0:c0 + Sc], dtype=np.float32)

    obs_p, yc_p, xc_p = pad_slice(obs), pad_slice(yc), pad_slice(xc)

    def tsh(a, dx):
        at = np.ascontiguousarray(a.T)
        if dx == 0:
            return at
        out = np.empty_like(at)
        if dx == -1:
            out[1:] = at[:-1]
            out[0] = at[0]
        else:
            out[:-1] = at[1:]
            out[-1] = at[-1]
        return out

    def cyc(n, d):
        # P[k, j] = 1 iff k == (j + d) mod n
        P = np.zeros((n, n), np.float32)
        P[(np.arange(n) + d) % n, np.arange(n)] = 1.0
        return P

    # sig: out free slot j <- cmapT partition sigma(j);
    # sigma(j) = j+1 for real cols, junk-col slot Dc -> partition 0 (1e7 lane)
    sigma = np.concatenate([np.arange(1, Sc), [0]])
    sigma[Dc] = 0
    sigma[Sc - 1] = Dc + 1
    assert sorted(sigma.tolist()) == list(range(Sc))
    sig = np.zeros((Sc, Sc), np.float32)
    sig[sigma, np.arange(Sc)] = 1.0

    def frame(a, pw):
        out = np.zeros((Sc, pw), np.float32)
        out[0:a.shape[0], 0:a.shape[1]] = a
        return out

    startm = np.zeros((Sc, Dc), np.float32)
    startm[0:Dr, :] = s[r0:r1 + 1, c0:c1 + 1]
    cr3 = np.concatenate([cyc(KR, -1), np.eye(KR, dtype=np.float32),
                          cyc(KR, 1)], axis=1)
    cc3 = np.concatenate([cyc(KC, -1), np.eye(KC, dtype=np.float32),
                          cyc(KC, 1)], axis=1)
    packed = np.concatenate([
        tsh(obs_p, 0), tsh(obs_p, -1), tsh(obs_p, 1),
        tsh(xc_p, 0), tsh(xc_p, -1), tsh(xc_p, 1), tsh(yc_p, 0),
        startm, np.eye(Sc, dtype=np.float32), sig,
        frame(cr3, 3 * KR), frame(cc3, 3 * KC),
    ], axis=1)
    in_map = {"packed": np.ascontiguousarray(packed, dtype=np.float32)}

    geom = dict(Dr=Dr, Dc=Dc, r0=r0, c0=c0, H=H, W=W,
                seed_rlo=int(ys.min()) - r0, seed_rhi=int(ys.max()) - r0,
                seed_clo=int(xs.min()) - c0, seed_chi=int(xs.max()) - c0)
    return in_map, geom


def kernel(obstacles, coords, start_map, goal_map):
    in_map, gm = prep_inputs(obstacles, coords, start_map)
    nc, _ = build_program(gm["Dr"], gm["Dc"], gm["seed_rlo"], gm["seed_rhi"],
                          gm["seed_clo"], gm["seed_chi"], gm["r0"], gm["c0"],
                          gm["H"], gm["W"], NUM_SWEEPS)
    in_maps = [in_map for _ in range(N_CORES)]
    res = run_bass_kernel_spmd(nc, in_maps, core_ids=list(range(N_CORES)))
    out = res.results[0]["out"]
    return np.ascontiguousarray(out[None, None]).astype(np.float32)
